# revision 1
# baseline (speedup 1.0000x reference)
"""Bidirectional Keras GRU (reset_after) on 8 Trainium2 NeuronCores.

B=64, T=512, D=H=768. SPMD: identical program on all 8 cores; core c<4 runs the
forward direction for batches 16c:16c+16, core c>=4 runs backward (inputs
pre-reversed in t on host, outputs flipped back). No collectives.

Per core (v2):
  - 3 concurrent PE column groups: batches live at psum partitions {0,32,64}+0:16,
    gate features split 3x256. The h@U recurrence matmuls for the three groups
    stream concurrently (tile_position=(0,32j)), tripling effective PE rate.
  - xp = x @ W + b staged in an SBUF ring (Phase A interleaved into the
    recurrence loop, one 384-col chunk per step) -- no DRAM round trip.
  - Per-step xp / bias adds are INJECTED into PSUM via tiny identity matmuls
    (K=16 ident / K=1 ones stationary), so gate pre-activations are complete in
    PSUM and sigmoid/tanh read PSUM directly on ScalarE.
  - blend uses 1-z = sigmoid(-zpre): h_new = z*h_prev + sigmoid(-zpre)*hh,
    two short DVE chains instead of one long one. Elementwise mostly bf16;
    h state f32 (PE transpose output dtype must match its input).
  - h_new^T rebuilt with 6 PE transposes, ONE row strip at a time: more than
    one in-flight transpose row strip wedges the device (HW quirk, found
    empirically); each strip's hT copy doubles as a psum-bank WAR serializer.
  - software pipelined: next step's xp/bias injects + r-gate matmuls are
    emitted in the transpose tail so the PE keeps streaming.
"""

import os
import numpy as np

import concourse.bass as bass
import concourse.bacc as bacc
import concourse.mybir as mybir
import concourse.tile as tile
from concourse.bass_utils import run_bass_kernel_spmd

B, D, H = 64, 768, 768
T = int(os.environ.get("GRU_T", "512"))
G3 = 3 * H          # 2304
NCORE = 8
BL = 16             # batches per core
KT = 6              # k-tiles over H/D (contraction)
NG = 3              # PE column groups (quadrant 3 unusable)
FG = H // NG        # 256 features per group per gate
AHEAD = 2           # Phase A prefetch distance (tci chunks)
SD = 4              # per-step staging DMA prefetch distance
RING = 4            # xpc ring depth

f32 = mybir.dt.float32
bf16 = mybir.dt.bfloat16
AF = mybir.ActivationFunctionType
OP = mybir.AluOpType


def build_program(t_steps=T):
    tc_n = t_steps // 8
    nc = bacc.Bacc("TRN2", target_bir_lowering=False, debug=False, num_devices=NCORE)

    x_prep = nc.dram_tensor("x_prep", (KT, tc_n, 128, 128), bf16, kind="ExternalInput")
    w_all = nc.dram_tensor("w_all", (KT, 128, G3), bf16, kind="ExternalInput")
    u_all = nc.dram_tensor("u_all", (KT, 128, G3), bf16, kind="ExternalInput")
    bfold_d = nc.dram_tensor("bfold_row", (1, G3), bf16, kind="ExternalInput")
    b1h_d = nc.dram_tensor("b1h_row", (1, H), bf16, kind="ExternalInput")
    identi_d = nc.dram_tensor("identI", (16, 16), bf16, kind="ExternalInput")
    identt_d = nc.dram_tensor("identT", (128, 16), f32, kind="ExternalInput")
    ones_d = nc.dram_tensor("ones_row", (1, 128), bf16, kind="ExternalInput")
    xp_dram = nc.dram_tensor("xp_scratch", (tc_n, 128, G3), bf16, kind="Internal")
    out_core = nc.dram_tensor("out_core", (t_steps, BL, H), f32, kind="ExternalOutput")

    with tile.TileContext(nc) as tctx:
        with (
            tctx.tile_pool(name="const", bufs=1) as cpool,
            tctx.tile_pool(name="xt", bufs=2 * KT) as xtpool,
            tctx.tile_pool(name="xpc", bufs=RING) as xpcpool,
            tctx.tile_pool(name="zrst", bufs=SD + 2) as zrpool,
            tctx.tile_pool(name="xhst", bufs=SD + 2) as xhpool,
            tctx.tile_pool(name="hs", bufs=3) as hpool,
            tctx.tile_pool(name="hT", bufs=2) as hTpool,
            tctx.tile_pool(name="gt", bufs=2) as gpool,
            tctx.tile_pool(name="psg", bufs=1, space="PSUM") as psg,
            tctx.tile_pool(name="psx", bufs=2, space="PSUM") as psx,
            tctx.tile_pool(name="pst", bufs=2, space="PSUM") as pst,
        ):
            u_sb = cpool.tile([128, KT, G3], bf16, tag="u")
            w_sb = cpool.tile([128, KT, G3], bf16, tag="w")
            bfold = cpool.tile([1, G3], bf16, tag="bf")
            b1h = cpool.tile([1, H], bf16, tag="b1h")
            identi = cpool.tile([16, 16], bf16, tag="idi")
            identt = cpool.tile([128, 16], f32, tag="idt")
            ones = cpool.tile([1, 128], bf16, tag="ones")

            nc.sync.dma_start(u_sb[:], u_all[:].rearrange("k p c -> p k c"))
            nc.sync.dma_start(w_sb[:], w_all[:].rearrange("k p c -> p k c"))
            nc.sync.dma_start(bfold[:], bfold_d[:])
            nc.sync.dma_start(b1h[:], b1h_d[:])
            nc.sync.dma_start(identi[:], identi_d[:])
            nc.sync.dma_start(identt[:], identt_d[:])
            nc.sync.dma_start(ones[:], ones_d[:])

            xpc_ring = [None] * RING
            xt_tiles = {}

            def emit_phase_a_loads(tci):
                tiles = []
                for k in range(KT):
                    xt = xtpool.tile([128, 128], bf16, tag="xt")
                    nc.sync.dma_start(xt[:], x_prep[k, tci])
                    tiles.append(xt)
                xt_tiles[tci] = tiles

            def emit_phase_a_chunk(tci, c):
                # one 384-col chunk of xp[tci] = x_t @ W + bfold -> xpc ring (bf16)
                if c == 0:
                    xpc_ring[tci % RING] = xpcpool.tile([128, G3], bf16, tag="xpc",
                                                        name=f"xpc{tci}")
                xpc = xpc_ring[tci % RING]
                ps = psx.tile([128, 384], f32, tag="psx")
                cs = 384 * c
                nc.tensor.matmul(ps[:], ones[0:1, :], bfold[0:1, cs:cs + 384],
                                 start=True, stop=False)
                for k in range(KT):
                    nc.tensor.matmul(ps[:], xt_tiles[tci][k][:],
                                     w_sb[:, k, cs:cs + 384],
                                     start=False, stop=(k == KT - 1))
                nc.vector.tensor_copy(xpc[:, cs:cs + 384], ps[:])
                nc.gpsimd.dma_start(xp_dram[tci, :, cs:cs + 384], xpc[:, cs:cs + 384])

            def emit_staging(t):
                # per-step xp slices staged from the DRAM scratch copy
                tci, dt = t // 8, t % 8
                zr = zrpool.tile([16, 2 * H], bf16, tag="zr")
                nc.sync.dma_start(zr[:], xp_dram[tci, 16 * dt:16 * dt + 16, 0:2 * H])
                xh = xhpool.tile([128, FG], bf16, tag="xh")
                for j in range(NG):
                    nc.sync.dma_start(
                        xh[32 * j:32 * j + 16, :],
                        xp_dram[tci, 16 * dt:16 * dt + 16,
                                2 * H + FG * j:2 * H + FG * j + FG])
                return zr, xh

            # ---------------- prologue ----------------------------------------
            for tci in range(min(AHEAD, tc_n)):
                emit_phase_a_loads(tci)
                for c in range(KT):
                    emit_phase_a_chunk(tci, c)
            stages = [emit_staging(t) for t in range(min(SD, t_steps))]

            h_prev = hpool.tile([128, FG], f32, tag="h")
            nc.vector.memset(h_prev[:], 0.0)
            hT_cur = hTpool.tile([128, KT, 16], bf16, tag="hT")
            nc.vector.memset(hT_cur[:], 0.0)

            # gate column base in u_sb / zr staging: z=0, r=1, h=2 (natural)
            def emit_injects(t, zr_t):
                # open the three gates' psum accumulation groups for step t by
                # injecting xp (z, r) / b1_h (h) via tiny matmuls
                psd = {}
                for g, tag in ((1, "psr"), (2, "psh"), (0, "psz")):
                    ps = psg.tile([128, FG], f32, tag=tag, name=f"{tag}{t}")
                    psd[g] = ps
                    for j in range(NG):
                        if g == 2:
                            lhs, rhs = ones[0:1, 0:16], b1h[0:1, FG * j:FG * j + FG]
                        else:
                            cb = H * (g == 1) + FG * j
                            lhs, rhs = identi[:], zr_t[0:16, cb:cb + FG]
                        nc.tensor.matmul(
                            ps[32 * j:32 * j + 16, :], lhs, rhs,
                            start=True, stop=False, tile_position=(0, 32 * j),
                            skip_group_check=True,
                        )
                return psd

            def emit_hu(ps, g, hT, ks):
                for k in ks:
                    for j in range(NG):
                        nc.tensor.matmul(
                            ps[32 * j:32 * j + 16, :],
                            hT[:, k, :],
                            u_sb[:, k, H * g + FG * j:H * g + FG * j + FG],
                            start=False, stop=(k == KT - 1),
                            tile_position=(0, 32 * j),
                            skip_group_check=True,
                        )

            pending = emit_injects(0, stages[0][0])
            emit_hu(pending[1], 1, hT_cur, range(KT))

            for t in range(t_steps):
                tci, dt = t // 8, t % 8
                zr, xh = stages[t]
                psd = pending

                # ---- r gate (matmuls already issued in the previous tail) ----
                r_t = gpool.tile([128, FG], bf16, tag="r")
                nc.scalar.activation(r_t[:], psd[1][:], AF.Sigmoid)

                # ---- h candidate ----
                emit_hu(psd[2], 2, hT_cur, range(KT))
                t2 = gpool.tile([128, FG], bf16, tag="t2")
                nc.vector.tensor_tensor(t2[:], psd[2][:], r_t[:], op=OP.mult)
                t3 = gpool.tile([128, FG], bf16, tag="t3")
                nc.vector.tensor_tensor(t3[:], t2[:], xh[:], op=OP.add)
                hh = gpool.tile([128, FG], bf16, tag="hh")
                nc.scalar.activation(hh[:], t3[:], AF.Tanh)

                # ---- z gate (and 1-z via sigmoid(-x)) ----
                emit_hu(psd[0], 0, hT_cur, range(KT))
                z_t = gpool.tile([128, FG], bf16, tag="z")
                nc.scalar.activation(z_t[:], psd[0][:], AF.Sigmoid)
                zc_t = gpool.tile([128, FG], bf16, tag="zc")
                nc.scalar.activation(zc_t[:], psd[0][:], AF.Sigmoid, scale=-1.0)

                # ---- blend: h_new = z*h_prev + (1-z)*hh ----
                a_t = gpool.tile([128, FG], f32, tag="a")
                nc.vector.tensor_tensor(a_t[:], z_t[:], h_prev[:], op=OP.mult)
                b_t = gpool.tile([128, FG], f32, tag="b")
                nc.vector.tensor_tensor(b_t[:], zc_t[:], hh[:], op=OP.mult)
                h_new = hpool.tile([128, FG], f32, tag="h")
                nc.vector.tensor_tensor(h_new[:], a_t[:], b_t[:], op=OP.add)

                for j in range(NG):
                    nc.gpsimd.dma_start(out_core[t, :, FG * j:FG * j + FG],
                                        h_new[32 * j:32 * j + 16, :])

                # ---- tail: rebuild h^T; next step's injects + r matmuls are
                # emitted here so the PE has work during the serialization ----
                # HW quirk: >1 in-flight transpose row strip wedges the device;
                # each strip's hT copy is a psum-bank WAR serializer for the
                # next strip's transposes.
                if t + 1 < t_steps:
                    tp = pst.tile([128, KT * 16], f32, tag="tp")
                    hT_new = hTpool.tile([128, KT, 16], bf16, tag="hT")
                    zr_next = stages[t + 1][0]
                    pend_new = None
                    for i, base in enumerate((0, 32, 64)):
                        for cc in range(2):
                            kt = 2 * i + cc
                            nc.tensor.transpose(
                                tp[:, 16 * kt:16 * kt + 16],
                                h_new[base:base + 16, 128 * cc:128 * cc + 128],
                                identt[base:base + 16, :],
                            )
                        nc.vector.tensor_copy(hT_new[:, 2 * i:2 * i + 2, :],
                                              tp[:, 32 * i:32 * i + 32])
                        if i == 0:
                            pend_new = emit_injects(t + 1, zr_next)
                    emit_hu(pend_new[1], 1, hT_new, range(KT))
                    pending = pend_new
                    hT_cur = hT_new

                # ---- prefetch: staging DMAs + interleaved Phase A ----
                if t + SD < t_steps:
                    stages.append(emit_staging(t + SD))
                if dt == 0 and tci + AHEAD < tc_n:
                    emit_phase_a_loads(tci + AHEAD)
                if dt < KT and tci + AHEAD < tc_n:
                    emit_phase_a_chunk(tci + AHEAD, dt)

                h_prev = h_new

    nc.compile()
    return nc


def _prep_core_inputs(x_c, Wd, Ud, bd, tc_n):
    """Host-side data prep for one core. x_c: [BL, t_steps, D] (already t-flipped
    for bwd cores)."""
    import ml_dtypes

    xp = np.ascontiguousarray(x_c.transpose(2, 1, 0))          # [D, T, BL]
    xp = xp.reshape(KT, 128, tc_n, 8, BL)
    x_prep = np.ascontiguousarray(xp.transpose(0, 2, 1, 3, 4)).reshape(KT, tc_n, 128, 128)

    w_all = np.ascontiguousarray(Wd.reshape(KT, 128, G3))
    u_all = np.ascontiguousarray(Ud.reshape(KT, 128, G3))

    zr_mask = np.zeros(G3, np.float32)
    zr_mask[0:2 * H] = 1.0
    bfold_row = (bd[0] + bd[1] * zr_mask).astype(np.float32).reshape(1, G3)
    b1h_row = bd[1][2 * H:].astype(np.float32).reshape(1, H)

    identT = np.zeros((128, 16), np.float32)
    for j in range(NG):
        identT[32 * j:32 * j + 16, :] = np.eye(16)
    ones_row = np.ones((1, 128), np.float32)

    bf = ml_dtypes.bfloat16
    return {
        "x_prep": x_prep.astype(bf),
        "w_all": w_all.astype(bf),
        "u_all": u_all.astype(bf),
        "bfold_row": bfold_row.astype(bf),
        "b1h_row": b1h_row.astype(bf),
        "identI": np.eye(16, dtype=np.float32).astype(bf),
        "identT": identT,
        "ones_row": ones_row.astype(bf),
    }


_NC_CACHE = {}
LAST_RESULT = None


def kernel(inputs, W_fwd, U_fwd, b_fwd, W_bwd, U_bwd, b_bwd, training=0):
    inputs = np.asarray(inputs, np.float32)
    t_steps = inputs.shape[1]
    tc_n = t_steps // 8
    if t_steps not in _NC_CACHE:
        _NC_CACHE[t_steps] = build_program(t_steps)
    nc = _NC_CACHE[t_steps]

    in_maps = []
    for c in range(NCORE):
        dirn = c // 4
        bs = slice(BL * (c % 4), BL * (c % 4) + BL)
        x_c = inputs[bs]
        if dirn:
            x_c = x_c[:, ::-1]
        Wd, Ud, bd = (W_fwd, U_fwd, b_fwd) if dirn == 0 else (W_bwd, U_bwd, b_bwd)
        in_maps.append(_prep_core_inputs(x_c, np.asarray(Wd, np.float32),
                                         np.asarray(Ud, np.float32),
                                         np.asarray(bd, np.float32), tc_n))

    trace = bool(os.environ.get("GRU_TRACE"))
    kw = {}
    if trace:
        kw = dict(trace=True, tmpdir=os.environ.get("GRU_TRACE_DIR", "/tmp/gru_trace"))
    res = run_bass_kernel_spmd(nc, in_maps, list(range(NCORE)), **kw)
    global LAST_RESULT
    LAST_RESULT = res
    if res.exec_time_ns is not None:
        print(f"HW exec time: {res.exec_time_ns} ns")

    out = np.empty((B, t_steps, 2 * H), np.float32)
    for c in range(NCORE):
        dirn = c // 4
        bs = slice(BL * (c % 4), BL * (c % 4) + BL)
        oc = np.asarray(res.results[c]["out_core"]).astype(np.float32)  # [T, BL, H]
        if dirn:
            oc = oc[::-1]
        out[bs, :, H * dirn:H * dirn + H] = oc.transpose(1, 0, 2)
    return out



# revision 18
# speedup vs baseline: 1.2441x; 1.2441x over previous
"""Bidirectional Keras GRU (reset_after) on 8 Trainium2 NeuronCores.

B=64, T=512, D=H=768. SPMD: identical program on all 8 cores; core c<4 runs the
forward direction for batches 16c:16c+16, core c>=4 runs backward (inputs
pre-reversed in t on host, outputs flipped back). No collectives.

v5 (vs the 3.77ms v2 baseline):
  - z|r gates merged into one N=512 matmul stream per (k, col-group) via a
    host-side column permutation of U/W.
  - xp staging DMAs and the DRAM scratch round-trip are gone: per-step xp /
    bias rows are injected into PSUM from SBUF by K=128 selector matmuls
    (selz has 16 zero columns so psum rows 0:96 are fully initialized; the
    recurrent h-bias uses a row-replicated constant so every inject is the
    SAME col-tiled K=128 config -> no PE tiling-mode drains).
  - h^T rebuilt with TWO full-width K=96 PE transposes (bf16) instead of six
    16-row strip transposes; separate PSUM banks so they can overlap.
  - every per-step tile (SBUF and PSUM) uses an explicit even/odd tag
    (tag=f"x{t%2}") => deterministic double buffering. The Tile scheduler's
    slot allocator otherwise reuses the just-released slot, chaining a WAR
    wait onto the CURRENT step's readers and serializing the pipeline.
  - Phase A (xp = x@W + b) runs one 288-col chunk every step on the PE; its
    psum->SBUF evacuation runs on the (mostly idle) Scalar engine so it can
    never head-block the DVE critical tail.
  - elementwise tail all bf16: split r/z sigmoids (ACT), zc=1-z on DVE,
    xh pre-staged to SBUF, blend a=z*h (GPSIMD), b=zc*hh, h=a+b (DVE).
"""

import os
import numpy as np

import concourse.bass as bass
import concourse.bacc as bacc
import concourse.mybir as mybir
import concourse.tile as tile
from concourse.bass_utils import run_bass_kernel_spmd

B, D, H = 64, 768, 768
T = int(os.environ.get("GRU_T", "512"))
G3 = 3 * H          # 2304
NCORE = 8
BL = 16             # batches per core
KT = 6              # k-tiles over H/D (contraction)
NG = 3              # PE column groups (quadrant 3 unusable)
FG = H // NG        # 256 features per group per gate
AHEAD = 2           # Phase A prefetch distance (tci = 8-step chunks)
XR = 3              # x tile ring depth
RING = 4            # xpc ring depth
NCHK = 8            # Phase A chunks per tci
CHK = G3 // NCHK    # 288 cols per chunk

f32 = mybir.dt.float32
bf16 = mybir.dt.bfloat16
AF = mybir.ActivationFunctionType
OP = mybir.AluOpType


def build_program(t_steps=T):
    tc_n = t_steps // 8
    nc = bacc.Bacc("TRN2", target_bir_lowering=False, debug=False, num_devices=NCORE)

    x_prep = nc.dram_tensor("x_prep", (tc_n, 128, KT * 128), bf16, kind="ExternalInput")
    w_all = nc.dram_tensor("w_all", (KT, 128, G3), bf16, kind="ExternalInput")
    u_all = nc.dram_tensor("u_all", (KT, 128, G3), bf16, kind="ExternalInput")
    bfold_d = nc.dram_tensor("bfold_row", (1, G3), bf16, kind="ExternalInput")
    b1hrep_d = nc.dram_tensor("b1h_rep", (128, H), bf16, kind="ExternalInput")
    ident_d = nc.dram_tensor("ident128", (128, 128), bf16, kind="ExternalInput")
    selz_d = nc.dram_tensor("selz", (128, 8 * 32), bf16, kind="ExternalInput")
    ones_d = nc.dram_tensor("ones_row", (1, 128), bf16, kind="ExternalInput")
    out_core = nc.dram_tensor("out_core", (t_steps, 128, 192), bf16,
                              kind="ExternalOutput")

    with tile.TileContext(nc) as tctx:
        with (
            tctx.tile_pool(name="const", bufs=1) as cpool,
            tctx.tile_pool(name="ring", bufs=1) as rpool,
            tctx.tile_pool(name="gt", bufs=1) as gpool,
            tctx.tile_pool(name="ps", bufs=1, space="PSUM") as pspool,
        ):
            u_sb = cpool.tile([128, KT, G3], bf16, tag="u")
            w_sb = cpool.tile([128, KT, G3], bf16, tag="w")
            bfold = cpool.tile([1, G3], bf16, tag="bf")
            b1h_rep = cpool.tile([128, H], bf16, tag="b1hr")
            ident = cpool.tile([128, 128], bf16, tag="id")
            selz = cpool.tile([128, 8, 32], bf16, tag="selz")
            ones = cpool.tile([1, 128], bf16, tag="ones")

            nc.sync.dma_start(u_sb[:], u_all[:].rearrange("k p c -> p k c"))
            nc.sync.dma_start(w_sb[:], w_all[:].rearrange("k p c -> p k c"))
            nc.sync.dma_start(bfold[:], bfold_d[:])
            nc.sync.dma_start(b1h_rep[:], b1hrep_d[:])
            nc.sync.dma_start(ident[:], ident_d[:])
            nc.sync.dma_start(selz[:], selz_d[:])
            nc.sync.dma_start(ones[:], ones_d[:])

            xsb_ring = [None] * XR
            xpc_ring = [None] * RING

            def load_x(tci):
                xt = rpool.tile([128, KT * 128], bf16, tag=f"xsb{tci % XR}",
                                name=f"xsb_{tci}")
                nc.sync.dma_start(xt[:], x_prep[tci])
                xsb_ring[tci % XR] = xt

            def phase_a_mms(tci, c):
                # one 288-col chunk of xp[tci] = x_t @ W + bfold (mms only)
                if c == 0:
                    xpc_ring[tci % RING] = rpool.tile([128, G3], bf16,
                                                      tag=f"xpc{tci % RING}",
                                                      name=f"xpc_{tci}")
                xt = xsb_ring[tci % XR]
                ps = pspool.tile([128, CHK], f32, tag=f"psx{c % 2}")
                cs = CHK * c
                nc.tensor.matmul(ps[:], ones[0:1, :], bfold[0:1, cs:cs + CHK],
                                 start=True, stop=False)
                for k in range(KT):
                    nc.tensor.matmul(ps[:], xt[:, 128 * k:128 * k + 128],
                                     w_sb[:, k, cs:cs + CHK],
                                     start=False, stop=(k == KT - 1))
                return ps

            def phase_a_evac(tci, c, ps):
                # on ACT: never competes with the DVE critical tail
                xpc = xpc_ring[tci % RING]
                cs = CHK * c
                nc.scalar.copy(xpc[:, cs:cs + CHK], ps[:])

            def emit_inject_zr(t):
                tci, dt = t // 8, t % 8
                xpc = xpc_ring[tci % RING]
                sel = selz[:, dt, :]
                ps_zr = pspool.tile([128, 2 * FG], f32, tag=f"pszr{t % 2}")
                for j in range(NG):
                    nc.tensor.matmul(
                        ps_zr[32 * j:32 * j + 32, :], sel,
                        xpc[:, 512 * j:512 * j + 512],
                        start=True, stop=False, tile_position=(0, 32 * j),
                        skip_group_check=True)
                return ps_zr

            def emit_inject_hx(t):
                tci, dt = t // 8, t % 8
                xpc = xpc_ring[tci % RING]
                sel = selz[:, dt, :]
                ps_hx = pspool.tile([128, 2 * FG], f32, tag=f"pshx{t % 2}")
                for j in range(NG):
                    # xh parked in cols 256:512 (own one-shot group)
                    nc.tensor.matmul(
                        ps_hx[32 * j:32 * j + 32, 256:512], sel,
                        xpc[:, 1536 + 256 * j:1536 + 256 * j + 256],
                        start=True, stop=True, tile_position=(0, 32 * j),
                        skip_group_check=True)
                    # recurrent h bias (row-replicated const -> same K=128
                    # col-tiled config as every other inject, no mode drain)
                    nc.tensor.matmul(
                        ps_hx[32 * j:32 * j + 32, 0:256], sel,
                        b1h_rep[:, 256 * j:256 * j + 256],
                        start=True, stop=False, tile_position=(0, 32 * j),
                        skip_group_check=True)
                return ps_hx

            def emit_mms(ps_zr, ps_hx, hTA, hTB):
                # recurrence matmuls; k-order {0,2,4} first so the stream can
                # start as soon as hTA (the first transpose's copy) lands
                for kt in (0, 2, 4, 1, 3, 5):
                    hT = hTA if kt % 2 == 0 else hTB
                    lhsT = hT[:, kt // 2, 0:16]
                    for j in range(NG):
                        nc.tensor.matmul(
                            ps_zr[32 * j:32 * j + 16, :], lhsT,
                            u_sb[:, kt, 512 * j:512 * j + 512],
                            start=False, stop=(kt == 5), tile_position=(0, 32 * j),
                            skip_group_check=True)
                for kt in (0, 2, 4, 1, 3, 5):
                    hT = hTA if kt % 2 == 0 else hTB
                    lhsT = hT[:, kt // 2, 0:16]
                    for j in range(NG):
                        nc.tensor.matmul(
                            ps_hx[32 * j:32 * j + 16, 0:256], lhsT,
                            u_sb[:, kt, 1536 + 256 * j:1536 + 256 * j + 256],
                            start=False, stop=(kt == 5), tile_position=(0, 32 * j),
                            skip_group_check=True)

            def emit_step_tail(t, ps_zr, ps_hx, h_prev):
                # gates + blend for step t: h = z*h_prev + (1-z)*hh, all bf16,
                # elementwise on partitions 0:96 (rows 96:128 unused).
                P = slice(0, 96)
                e = t % 2
                r_sb = gpool.tile([128, FG], bf16, tag=f"r{e}")
                nc.scalar.activation(r_sb[P], ps_zr[P, 256:512], AF.Sigmoid)
                z_sb = gpool.tile([128, FG], bf16, tag=f"z{e}")
                nc.scalar.activation(z_sb[P], ps_zr[P, 0:256], AF.Sigmoid)
                t2 = gpool.tile([128, FG], bf16, tag=f"t2{e}")
                nc.vector.tensor_tensor(t2[P], ps_hx[P, 0:256], r_sb[P], op=OP.mult)
                t3 = gpool.tile([128, FG], bf16, tag=f"t3{e}")
                nc.vector.tensor_tensor(t3[P], ps_hx[P, 256:512], t2[P], op=OP.add)
                zc = gpool.tile([128, FG], bf16, tag=f"zc{e}")
                nc.scalar.activation(zc[P], ps_zr[P, 0:256], AF.Sigmoid, scale=-1.0)
                hh = gpool.tile([128, FG], bf16, tag=f"hh{e}")
                nc.scalar.activation(hh[P], t3[P], AF.Tanh)
                a_t = gpool.tile([128, FG], bf16, tag=f"a{e}")
                nc.gpsimd.tensor_tensor(a_t[P], z_sb[P], h_prev[P], op=OP.mult)
                b_t = gpool.tile([128, FG], bf16, tag=f"b{e}")
                nc.vector.tensor_tensor(b_t[P], zc[P], hh[P], op=OP.mult)
                h_bf = gpool.tile([128, FG], bf16, tag=f"hbf{t % 3}")
                nc.vector.tensor_tensor(h_bf[P], a_t[P], b_t[P], op=OP.add)
                return h_bf

            def emit_transpose(t, h_bf):
                # hTA[p, j', b] = h[b, 256j' + p] (k-strips 0,2,4);
                # hTB: +128 (strips 1,3,5). K=96 transposes touch only the
                # initialized partitions; separate psum banks so the two
                # transposes can be in flight together.
                e = t % 2
                hTA = gpool.tile([128, 3, 32], bf16, tag=f"hTA{e}")
                hTB = gpool.tile([128, 3, 32], bf16, tag=f"hTB{e}")
                # PSUM budget is exactly 8 banks: pszr0/1, pshx0/1, psx0/1,
                # tpa, tpb. tpa/tpb carry no parity: the WAR against the
                # previous step's copy resolves a full period earlier.
                tpa = pspool.tile([128, 96], bf16, tag="tpa")
                tpb = pspool.tile([128, 96], bf16, tag="tpb")
                nc.tensor.transpose(tpa[:], h_bf[0:96, 0:128], ident[0:96, 0:96])
                nc.vector.tensor_copy(hTA[:], tpa[:])
                nc.tensor.transpose(tpb[:], h_bf[0:96, 128:256], ident[0:96, 0:96])
                nc.vector.tensor_copy(hTB[:], tpb[:])
                nc.sync.dma_start(out_core[t, :, 0:96], hTA[:])
                nc.sync.dma_start(out_core[t, :, 96:192], hTB[:])
                return hTA, hTB

            # ---------------- prologue ----------------------------------------
            for tci in range(min(XR, tc_n)):
                load_x(tci)
            for tci in range(min(AHEAD, tc_n)):
                for c in range(NCHK):
                    ps = phase_a_mms(tci, c)
                    phase_a_evac(tci, c, ps)

            h_prev = gpool.tile([128, FG], bf16, tag="hbf2")
            nc.vector.memset(h_prev[:], 0.0)
            hTA_cur = gpool.tile([128, 3, 32], bf16, tag="hTAi")
            nc.vector.memset(hTA_cur[:], 0.0)
            hTB_cur = gpool.tile([128, 3, 32], bf16, tag="hTBi")
            nc.vector.memset(hTB_cur[:], 0.0)

            pending = (emit_inject_zr(0), emit_inject_hx(0))
            emit_mms(pending[0], pending[1], hTA_cur, hTB_cur)
            pending_evac = None

            for t in range(t_steps):
                tci, dt = t // 8, t % 8
                ps_zr, ps_hx = pending

                pa = None
                if t + 1 < t_steps:
                    zr_next = emit_inject_zr(t + 1)
                if tci + AHEAD < tc_n:
                    pa = phase_a_mms(tci + AHEAD, dt)
                    if dt == 0 and tci + AHEAD + 1 < tc_n:
                        load_x(tci + AHEAD + 1)
                if t + 1 < t_steps:
                    hx_next = emit_inject_hx(t + 1)
                    pending = (zr_next, hx_next)

                h_bf = emit_step_tail(t, ps_zr, ps_hx, h_prev)
                hTA, hTB = emit_transpose(t, h_bf)
                if t + 1 < t_steps:
                    emit_mms(pending[0], pending[1], hTA, hTB)
                # evac AFTER the copies in DVE priority order: DVE idles here
                # (next step's t2 waits ~2us for the gate matmuls), and the
                # psx slot frees a full step before chunk dt+2 needs it.
                if pa is not None:
                    phase_a_evac(tci + AHEAD, dt, pa)
                h_prev = h_bf

    nc.compile()
    return nc


def _selz():
    """[128, 8*32]: selz[p, 32*dt + m] = 1 if m < 16 and p == 16*dt + m."""
    s = np.zeros((128, 8, 32), np.float32)
    for dt in range(8):
        for m in range(16):
            s[16 * dt + m, dt, m] = 1.0
    return s.reshape(128, 8 * 32)


def _gate_perm():
    perm = np.empty(G3, np.int64)
    ar = np.arange(FG)
    for j in range(NG):
        perm[512 * j:512 * j + 256] = ar + FG * j            # z block
        perm[512 * j + 256:512 * j + 512] = ar + H + FG * j  # r block
        perm[1536 + 256 * j:1536 + 256 * j + 256] = ar + 2 * H + FG * j  # h block
    return perm


def _prep_core_inputs(x_c, Wd, Ud, bd, tc_n):
    """Host-side data prep for one core. x_c: [BL, t_steps, D] (already t-flipped
    for bwd cores)."""
    import ml_dtypes

    perm = _gate_perm()

    xp = np.ascontiguousarray(x_c.transpose(2, 1, 0))          # [D, T, BL]
    x_prep = np.ascontiguousarray(
        xp.reshape(KT, 128, tc_n, 8, BL).transpose(2, 1, 0, 3, 4)
    ).reshape(tc_n, 128, KT * 128)

    w_all = np.ascontiguousarray(Wd[:, perm].reshape(KT, 128, G3))
    u_all = np.ascontiguousarray(Ud[:, perm].reshape(KT, 128, G3))

    zr_mask = np.zeros(G3, np.float32)
    zr_mask[0:2 * H] = 1.0
    bfold_row = (bd[0] + bd[1] * zr_mask)[perm].astype(np.float32).reshape(1, G3)
    b1h_rep = np.broadcast_to(bd[1][2 * H:].astype(np.float32), (128, H))

    bf = ml_dtypes.bfloat16
    return {
        "x_prep": x_prep.astype(bf),
        "w_all": w_all.astype(bf),
        "u_all": u_all.astype(bf),
        "bfold_row": bfold_row.astype(bf),
        "b1h_rep": np.ascontiguousarray(b1h_rep).astype(bf),
        "ident128": np.eye(128, dtype=np.float32).astype(bf),
        "selz": _selz().astype(bf),
        "ones_row": np.ones((1, 128), np.float32).astype(bf),
    }


def _decode_out(oc, t_steps):
    """[T, 128, 192] bf16 -> [BL, T, H] f32; f = 256j + 128c2 + p."""
    v = np.asarray(oc).astype(np.float32)
    v = v.reshape(t_steps, 128, 2, 3, 32)[:, :, :, :, 0:16]   # [T, p, c2, j, b]
    v = v.transpose(0, 3, 2, 1, 4).reshape(t_steps, H, BL)    # [T, f, b]
    return v.transpose(2, 0, 1)                               # [b, T, f]


_NC_CACHE = {}
LAST_RESULT = None


def kernel(inputs, W_fwd, U_fwd, b_fwd, W_bwd, U_bwd, b_bwd, training=0):
    inputs = np.asarray(inputs, np.float32)
    t_steps = inputs.shape[1]
    tc_n = t_steps // 8
    if t_steps not in _NC_CACHE:
        _NC_CACHE[t_steps] = build_program(t_steps)
    nc = _NC_CACHE[t_steps]

    in_maps = []
    for c in range(NCORE):
        dirn = c // 4
        bs = slice(BL * (c % 4), BL * (c % 4) + BL)
        x_c = inputs[bs]
        if dirn:
            x_c = x_c[:, ::-1]
        Wd, Ud, bd = (W_fwd, U_fwd, b_fwd) if dirn == 0 else (W_bwd, U_bwd, b_bwd)
        in_maps.append(_prep_core_inputs(x_c, np.asarray(Wd, np.float32),
                                         np.asarray(Ud, np.float32),
                                         np.asarray(bd, np.float32), tc_n))

    trace = bool(os.environ.get("GRU_TRACE"))
    kw = {}
    if trace:
        kw = dict(trace=True, tmpdir=os.environ.get("GRU_TRACE_DIR", "/tmp/gru_trace"))
    res = run_bass_kernel_spmd(nc, in_maps, list(range(NCORE)), **kw)
    global LAST_RESULT
    LAST_RESULT = res
    if res.exec_time_ns is not None:
        print(f"HW exec time: {res.exec_time_ns} ns")

    out = np.empty((B, t_steps, 2 * H), np.float32)
    for c in range(NCORE):
        dirn = c // 4
        bs = slice(BL * (c % 4), BL * (c % 4) + BL)
        oc = np.asarray(res.results[c]["out_core"])
        if dirn:
            oc = oc[::-1]
        out[bs, :, H * dirn:H * dirn + H] = _decode_out(oc, t_steps)
    return out
